# revision 6
# baseline (speedup 1.0000x reference)
"""Distributed Trainium2 (Bass/Tile) kernel for single-head latent attention.

Reference computation (B=4, S=4096, D=1024, DL=64):
    qkv = x @ Wd + bd; q,k,v = split(qkv)
    logits = (q @ k^T) / sqrt(DL) / TEMP, key-masked
    out = softmax(logits) @ v @ Wu + bu

Sharding: data-parallel over (batch, seq-half) -> 8 shards of 2048 query rows.
Each core recomputes K/V for its batch's keys from x (no collectives).

Key tricks (carried from the 2-pass baseline):
  - Host-side mask compaction: only unmasked rows (~2040 of 4096, capped at
    K_CAP=2176) are gathered as keys. Pad slots get exp-bias -1e30.
  - Softmax without row-max: scaled logits are bounded, shifted by -40 in the
    exp bias, so flash accumulation over key chunks is plain PSUM accumulation.
  - PV lhsT is [ones | v] [128, 65]: accumulator row 0 is Z, rows 1:65 ctxU.
    After normalizing by broadcast(1/Z), row 0 is exactly 1.0 and the
    up-projection rhs [bu; Wu] folds in the bias.
  - dtypes: x/Wd/q/k/Wu fp16, exp/v bf16, out f16.

Schedule (v2): FOUR 512-col attention passes instead of two 1024 ones.
  - Narrow pass 0 starts after only wd + xq slab0 + xk range0 (~1.9MB) land
    (kv range 0 shrunk to 256 keys for an earlier start); narrow pass 3
    leaves only a ~5us tail (norm + 4 up tiles + 1MB out).
  - PE clock warmup (HAM 1.2->2.4GHz trips after ~6us of dense matmuls) runs
    on a memset SBUF tile starting at t~0, with no DMA dependency.
  - Inputs are released in consumption order via gated groups (a DVE memset
    on the dst creates a WAW dep that throttles the sync queue; concurrent
    DMAs on the ring progress at equal rates). The first gate is timed by a
    DVE delay-chain memset; later gates ride the in-order DVE queue.
  - Extras (kv ranges 1-5, V transposes, next-pass q projection, previous
    pass's normalize + up tiles) ride each pass's PE slack; psum evacuation
    is all on DVE so ACT runs the exp stream (68 x [128,512]) unobstructed.
"""

import sys

if "/opt/trn_rl_repo" not in sys.path:
    sys.path.insert(0, "/opt/trn_rl_repo")

import numpy as np

from concourse import bacc, tile
from concourse import mybir
from concourse.masks import make_identity

F32 = mybir.dt.float32
F32R = mybir.dt.float32r
BF16 = mybir.dt.bfloat16
F16 = mybir.dt.float16

B, S, D, DL = 4, 4096, 1024, 64
N_CORES = 8
S_LOC = S // 2          # 2048 query rows per core
QW = 512                # attention pass width (q cols per pass)
NP = 4                  # number of passes
JC = 128                # key chunk
NJK = 17                # compacted key chunks
K_CAP = NJK * JC        # 2176 >= max unmasked keys per batch
VB = 80                 # v_aug block stride: [pad(15) | ones(1) | v(64)]
SCALE = 1.25            # 1/sqrt(64)/0.1
LOGIT_SHIFT = -40.0
MASKED_BIAS = -1e30
N_WARM = 22             # clock-warmup dummy matmuls
DELAY_COLS = 3968       # DVE delay-chain cols timing the first input gate

# key ranges for the kv projection: 2x256 + 3x512 + 128
KV_RANGES = [(0, 256), (256, 256), (512, 512), (1024, 512), (1536, 512),
             (2048, 128)]

_CACHE = {}


def build_graph():
    """Core-agnostic Bacc graph; each core's inputs are pre-sliced host-side
    (local query half + compacted keys of its batch, in contiguous slabs)."""
    nc = bacc.Bacc("TRN2", target_bir_lowering=False, debug=False,
                   num_devices=N_CORES)

    # xq: [128, slab(4) x 8 x 512] (512-row q slabs, d-chunk-major inside)
    xq_d = nc.dram_tensor("xq", [128, NP * 8 * QW], F16, kind="ExternalInput").ap()
    xk_d = nc.dram_tensor("xk", [128, 8 * K_CAP], F16, kind="ExternalInput").ap()
    wd_d = nc.dram_tensor("Wd", [128, 8 * 192], F16, kind="ExternalInput").ap()
    wub_d = nc.dram_tensor("Wub", [DL + 1, D], F16, kind="ExternalInput").ap()
    bdq_d = nc.dram_tensor("bd_q", [64, 1], F32, kind="ExternalInput").ap()
    bdkv_d = nc.dram_tensor("bd_kv", [128, 1], F32, kind="ExternalInput").ap()
    mb_d = nc.dram_tensor("maskbias", [128, NJK], F32, kind="ExternalInput").ap()
    out_d = nc.dram_tensor("out", [S_LOC, D], F16, kind="ExternalOutput").ap()

    with tile.TileContext(nc) as tc, nc.allow_low_precision(
            reason="bf16/f16 tiles feed full-rate PE matmuls; ~10-bit "
                   "mantissas are far inside the 2e-2 error budget"):
        with (
            tc.tile_pool(name="consts", bufs=1) as consts,
            tc.tile_pool(name="acts", bufs=1) as acts,
            tc.tile_pool(name="ep", bufs=6) as ep,
            tc.tile_pool(name="ob", bufs=4) as ob,
        ):
            # ---- clock warmup input: no DMA dependency ---------------------
            warm_in = consts.tile([128, QW], F16)
            nc.vector.memset(warm_in[:], 1.0)

            # ---- DMA plan --------------------------------------------------
            wd_s = consts.tile([128, 8 * 192], F16)
            nc.sync.dma_start(out=wd_s[:], in_=wd_d[:])
            bdq_s = consts.tile([64, 1], F32)
            nc.sync.dma_start(out=bdq_s[:], in_=bdq_d[:])
            bdkv_s = consts.tile([128, 1], F32)
            nc.sync.dma_start(out=bdkv_s[:], in_=bdkv_d[:])
            mb_s = consts.tile([128, NJK], F32)
            nc.sync.dma_start(out=mb_s[:], in_=mb_d[:])
            # preload the exp ACT table set early so the table-load stall
            # doesn't hit the exp stream at attention start
            act_warm = consts.tile([128, NJK], F32)
            nc.scalar.activation(act_warm[:], mb_s[:],
                                 mybir.ActivationFunctionType.Exp)
            ones_colf = consts.tile([1, 128], F32)
            nc.vector.memset(ones_colf[:], 1.0)
            ones_col = consts.tile([1, 128], F32R)
            nc.vector.tensor_copy(ones_col[:], ones_colf[:])
            # identity at partitions 64:128 (vT rows live there), bf16
            ident2f = consts.tile([128, 64], F32)
            nc.vector.memset(ident2f[:], 0.0)
            make_identity(nc, ident2f[64:128, :], nomemset=True)
            ident2 = consts.tile([128, 64], BF16)
            nc.vector.tensor_copy(ident2[:], ident2f[:])

            xq_sb = acts.tile([128, NP * 8 * QW], F16)
            xk_sb = acts.tile([128, 8 * K_CAP], F16)

            # Concurrently-outstanding DMAs on the ring progress at EQUAL
            # rates (SDMA packet round-robin), so transfers are released in
            # gated groups matching consumption order.
            def xk_range_dma(r):
                c0, w = KV_RANGES[r]
                nc.sync.dma_start(out=xk_sb[:, 8 * c0:8 * (c0 + w)],
                                  in_=xk_d[:, 8 * c0:8 * (c0 + w)])

            def xk_gate(r):
                c0, w = KV_RANGES[r]
                nc.vector.memset(xk_sb[:, 8 * c0:8 * c0 + 1], 0.0)

            def xq_dma(p):
                sl = slice(p * 8 * QW, (p + 1) * 8 * QW)
                nc.sync.dma_start(out=xq_sb[:, sl], in_=xq_d[:, sl])

            def xq_gate(p):
                nc.vector.memset(xq_sb[:, p * 8 * QW:p * 8 * QW + 1], 0.0)

            # group 1: exactly what pass 0 needs to start
            xq_dma(0)
            xk_range_dma(0)
            wub_s = consts.tile([DL + 1, D], F16)

            qT_s = acts.tile([64, S_LOC], F16)
            kT_s = acts.tile([64, K_CAP], F16)
            # vT (projection layout [dl, keys]) at partitions 64:128, bf16
            vT_hi = acts.tile([128, K_CAP], BF16)
            # PV stationary per key chunk: col +15 = ones, cols +16:+80 = v
            v_aug = acts.tile([128, NJK * VB], BF16)
            nc.vector.memset(v_aug[:], 1.0)
            # DVE delay chain: times the release of xk ranges 1-2 to land
            # just after group 1 without stealing its ring bandwidth
            delay_scr = acts.tile([128, DELAY_COLS], F16)
            nc.vector.memset(delay_scr[:], 0.0)
            ctxu_s = acts.tile([DL + 1, S_LOC], F32R)
            rzb_s = acts.tile([DL + 1, S_LOC], F32)
            rzb_scr = acts.tile([DL + 1, S_LOC], F32)
            ctxn_s = acts.tile([DL + 1, S_LOC], F16)

            # early-released inputs, timed by the delay chain
            xk_gate(1)
            xk_gate(2)
            xk_range_dma(1)
            xk_range_dma(2)

            # PSUM budget is exactly 8 banks:
            #   pl 2x[128,512] = 2, pc 2x[65,512] = 2,
            #   pp 2x[128,512] = 2, po 2x[128,512] = 2
            with (
                tc.tile_pool(name="pl", bufs=2, space="PSUM") as pl,
                tc.tile_pool(name="pc", bufs=2, space="PSUM") as pc,
                tc.tile_pool(name="pp", bufs=2, space="PSUM") as pp,
                tc.tile_pool(name="po", bufs=2, space="PSUM") as po,
            ):
                # ---- helpers -----------------------------------------------
                def q_col(p, k):
                    return p * 8 * QW + k * QW

                def q_proj_mms(p, ps, k0, k1):
                    for k in range(k0, k1):
                        nc.tensor.matmul(
                            ps[:], wd_s[:, k * 192:k * 192 + 64],
                            xq_sb[:, q_col(p, k):q_col(p, k) + QW],
                            start=(k == 0), stop=(k == 7))

                def q_bias(p, ps):
                    nc.vector.tensor_scalar_add(
                        qT_s[:, p * QW:(p + 1) * QW], ps[:64, :], bdq_s[:])

                def kv_mms(r, ps, ks):
                    c0, w = KV_RANGES[r]
                    for k in ks:
                        nc.tensor.matmul(
                            ps[:, 0:w], wd_s[:, k * 192 + 64:(k + 1) * 192],
                            xk_sb[:, 8 * c0 + k * w:8 * c0 + (k + 1) * w],
                            start=(k == 0), stop=(k == 7))

                def kv_bias(r, ps):
                    c0, w = KV_RANGES[r]
                    nc.vector.tensor_scalar_add(kT_s[:, c0:c0 + w],
                                                ps[0:64, 0:w],
                                                bdkv_s[0:64, :])
                    nc.vector.tensor_scalar_add(vT_hi[64:128, c0:c0 + w],
                                                ps[64:128, 0:w],
                                                bdkv_s[64:128, :])

                def v_transpose(c):
                    # [dl, keys] -> [keys, dl] on the PE (bf16 psum)
                    vt = pp.tile([128, 64], BF16, tag="p", name=f"vt{c}")
                    nc.tensor.transpose(
                        vt[:], vT_hi[64:128, c * JC:(c + 1) * JC],
                        ident2[64:128, :])
                    nc.vector.tensor_copy(
                        v_aug[:, c * VB + 16:c * VB + 80], vt[:])

                def v_transpose_dma(c):
                    # mid-deadline chunks ride the sync queue via the XBAR
                    nc.sync.dma_start(
                        out=v_aug[:, c * VB + 16:c * VB + 80],
                        in_=vT_hi[64:128, c * JC:(c + 1) * JC],
                        transpose=True)

                # ---- head: warmup + pass-0 q + kv range 0 ------------------
                # HAM clock warmup: dummy matmuls on a memset tile starting
                # at t~0 trip the PE to 2.4 GHz (~6us of dense work) before
                # the first real projections run.
                warm_ps = pp.tile([128, QW], F32, tag="p", name="warm_ps")
                for _ in range(N_WARM):
                    nc.tensor.matmul(warm_ps[:], warm_in[:, 0:128],
                                     warm_in[:, 0:QW], start=True, stop=True)

                ps_q0 = pp.tile([64, QW], F32, tag="p", name="ps_q0")
                q_proj_mms(0, ps_q0, 0, 8)
                q_bias(0, ps_q0)
                ps_kv0 = pp.tile([128, KV_RANGES[0][1]], F32, tag="p",
                                 name="pskv0")
                kv_mms(0, ps_kv0, range(8))
                kv_bias(0, ps_kv0)
                v_transpose(0)
                v_transpose(1)

                # ---- attention passes with interleaved extras --------------
                kv_ps = {}

                def kv_part(r, k0, k1, bias=False):
                    def f():
                        if k0 == 0:
                            kv_ps[r] = pp.tile([128, KV_RANGES[r][1]], F32,
                                               tag="p", name=f"pskv{r}")
                        kv_mms(r, kv_ps[r], range(k0, k1))
                        if bias:
                            kv_bias(r, kv_ps[r])
                    return f

                def vts(*cs):
                    def f():
                        for c in cs:
                            v_transpose(c)
                    return f

                def vts_dma(*cs):
                    def f():
                        for c in cs:
                            v_transpose_dma(c)
                    return f

                qb_ps = {}

                def qproj_part(p, k0, k1):
                    def f():
                        if k0 == 0:
                            qb_ps[p] = pp.tile([64, QW], F32, tag="p",
                                               name=f"psqb{p}")
                        q_proj_mms(p, qb_ps[p], k0, k1)
                        if k1 == 8:
                            q_bias(p, qb_ps[p])
                    return f

                def gate(xks=(), xqs=(), wub=False):
                    def f():
                        for r in xks:
                            xk_gate(r)
                        for p in xqs:
                            xq_gate(p)
                        if wub:
                            nc.vector.memset(wub_s[:, 0:1], 0.0)
                        for r in xks:
                            xk_range_dma(r)
                        for p in xqs:
                            xq_dma(p)
                        if wub:
                            nc.sync.dma_start(out=wub_s[:], in_=wub_d[:])
                    return f

                # ---- per-pass epilogue pieces (ride the NEXT pass) ---------
                ctx_tiles = {}
                exs = {}

                def ctx_evac(p):
                    def f():
                        sl = slice(p * QW, (p + 1) * QW)
                        nc.vector.tensor_copy(ctxu_s[:, sl],
                                              ctx_tiles[p][:, :])
                    return f

                def norm(p):
                    # broadcast Z (ctx row 0) across 65 rows, recip, scale
                    def f():
                        sl = slice(p * QW, (p + 1) * QW)
                        zb = po.tile([DL + 1, QW], F32, tag="o",
                                     name=f"zb{p}")
                        nc.tensor.matmul(zb[:], ones_col[:, 0:DL + 1],
                                         ctxu_s[0:1, sl],
                                         start=True, stop=True)
                        nc.vector.reciprocal_approx_accurate(
                            rzb_s[:, sl], zb[:], rzb_scr[:, sl])
                        nc.vector.tensor_mul(ctxn_s[:, sl], ctxu_s[:, sl],
                                             rzb_s[:, sl])
                    return f

                osbs = {}

                def up_half(p, t, s2):
                    # one [128,512] up-proj matmul, DVE-evacuated; the DMA
                    # goes out once both halves of the [128,1024] tile land
                    def f():
                        st = p * 4 + t
                        if s2 == 0:
                            osbs[st] = ob.tile([128, D], F16, tag="ot",
                                               name=f"osb{st}")
                        up = po.tile([128, QW], F32, tag="o",
                                     name=f"up{st}_{s2}")
                        nc.tensor.matmul(
                            up[:],
                            ctxn_s[:, st * 128:(st + 1) * 128],
                            wub_s[:, s2 * QW:(s2 + 1) * QW],
                            start=True, stop=True)
                        nc.vector.tensor_copy(
                            osbs[st][:, s2 * QW:(s2 + 1) * QW], up[:])
                        if s2 == 1:
                            nc.sync.dma_start(
                                out=out_d[st * 128:(st + 1) * 128, :],
                                in_=osbs[st][:])
                    return f

                def mm1_exp(p, c):
                    lg = pl.tile([128, QW], F32, tag="l", name=f"lg{p}_{c}")
                    nc.tensor.matmul(
                        lg[:], kT_s[:, c * JC:(c + 1) * JC],
                        qT_s[:, p * QW:(p + 1) * QW],
                        start=True, stop=True)
                    ex = ep.tile([128, QW], BF16, tag="e", name=f"ex{p}_{c}")
                    nc.scalar.activation(
                        ex[:], lg[:], mybir.ActivationFunctionType.Exp,
                        bias=mb_s[:, c:c + 1], scale=SCALE)
                    exs[(p, c)] = ex

                def mm2(p, c):
                    nc.tensor.matmul(
                        ctx_tiles[p][:, :],
                        v_aug[:, c * VB + 15:c * VB + 80],
                        exs[(p, c)][:, :],
                        start=(c == 0), stop=(c == NJK - 1))
                    del exs[(p, c)]

                # extras[(p, c)] run right before MM1(p, c).
                extras = {
                    # pass 0: kv ranges 1-5, V transposes, q slab 1
                    (0, 1): [kv_part(1, 0, 8, bias=True)],
                    (0, 2): [vts(2, 3), gate(xks=(3,))],
                    (0, 3): [kv_part(2, 0, 4)],
                    (0, 4): [kv_part(2, 4, 8, bias=True)],
                    (0, 5): [vts(4, 5), gate(xks=(4,), xqs=(1,), wub=True)],
                    (0, 6): [vts(6, 7)],
                    (0, 7): [kv_part(3, 0, 4)],
                    (0, 8): [kv_part(3, 4, 8, bias=True), gate(xks=(5,))],
                    (0, 9): [vts(8, 9)],
                    (0, 10): [vts_dma(10, 11)],
                    (0, 11): [kv_part(4, 0, 4)],
                    (0, 12): [kv_part(4, 4, 8, bias=True), vts(12),
                              gate(xqs=(2,))],
                    (0, 13): [vts(13, 14)],
                    (0, 14): [kv_part(5, 0, 8, bias=True), vts(15),
                              qproj_part(1, 0, 3)],
                    (0, 15): [vts(16), qproj_part(1, 3, 6)],
                    (0, 16): [qproj_part(1, 6, 8)],
                    # pass 1: pass-0 epilogue + q slab 2
                    (1, 1): [ctx_evac(0)],
                    (1, 2): [norm(0)],
                    (1, 3): [gate(xqs=(3,))],
                    (1, 4): [up_half(0, 0, 0)],
                    (1, 5): [up_half(0, 0, 1)],
                    (1, 6): [up_half(0, 1, 0)],
                    (1, 7): [up_half(0, 1, 1)],
                    (1, 8): [up_half(0, 2, 0)],
                    (1, 9): [up_half(0, 2, 1)],
                    (1, 10): [up_half(0, 3, 0)],
                    (1, 11): [up_half(0, 3, 1)],
                    (1, 13): [qproj_part(2, 0, 3)],
                    (1, 14): [qproj_part(2, 3, 6)],
                    (1, 15): [qproj_part(2, 6, 8)],
                    # pass 2: pass-1 epilogue + q slab 3
                    (2, 1): [ctx_evac(1)],
                    (2, 2): [norm(1)],
                    (2, 4): [up_half(1, 0, 0)],
                    (2, 5): [up_half(1, 0, 1)],
                    (2, 6): [up_half(1, 1, 0)],
                    (2, 7): [up_half(1, 1, 1)],
                    (2, 8): [up_half(1, 2, 0)],
                    (2, 9): [up_half(1, 2, 1)],
                    (2, 10): [up_half(1, 3, 0)],
                    (2, 11): [up_half(1, 3, 1)],
                    (2, 13): [qproj_part(3, 0, 3)],
                    (2, 14): [qproj_part(3, 3, 6)],
                    (2, 15): [qproj_part(3, 6, 8)],
                    # pass 3: pass-2 epilogue
                    (3, 1): [ctx_evac(2)],
                    (3, 2): [norm(2)],
                    (3, 4): [up_half(2, 0, 0)],
                    (3, 5): [up_half(2, 0, 1)],
                    (3, 6): [up_half(2, 1, 0)],
                    (3, 7): [up_half(2, 1, 1)],
                    (3, 8): [up_half(2, 2, 0)],
                    (3, 9): [up_half(2, 2, 1)],
                    (3, 10): [up_half(2, 3, 0)],
                    (3, 11): [up_half(2, 3, 1)],
                }

                for p in range(NP):
                    ctx_tiles[p] = pc.tile([DL + 1, QW], F32, tag="c",
                                           name=f"ctx{p}")
                    for c in range(NJK):
                        # previous pass's MM2 tail rides the seam
                        if c == 0 and p > 0:
                            mm2(p - 1, NJK - 2)
                        if c == 1 and p > 0:
                            mm2(p - 1, NJK - 1)
                        for f in extras.get((p, c), ()):
                            f()
                        mm1_exp(p, c)
                        if c >= 2:
                            mm2(p, c - 2)

                # ---- tail: pass-3 epilogue ---------------------------------
                mm2(3, NJK - 2)
                mm2(3, NJK - 1)
                ctx_evac(3)()
                norm(3)()
                for t in range(4):
                    up_half(3, t, 0)()
                    up_half(3, t, 1)()

    nc.compile()
    return nc


def get_graph():
    if "graph" not in _CACHE:
        _CACHE["graph"] = build_graph()
    return _CACHE["graph"]


def make_in_maps(x, attention_mask, Wd, bd, Wu, bu):
    # up-proj rhs [bu; Wu]: bias row first (ctx row 0 is the Z/ones row)
    wub = np.ascontiguousarray(
        np.concatenate([bu[None, :], Wu], axis=0).astype(np.float16))
    wd_c = np.ascontiguousarray(
        Wd.astype(np.float16).reshape(8, 128, 192).transpose(1, 0, 2)
        .reshape(128, 8 * 192))
    bd_q = np.ascontiguousarray(bd[0:64].reshape(64, 1).astype(np.float32))
    bd_kv = np.ascontiguousarray(bd[64:192].reshape(128, 1).astype(np.float32))
    per_batch = []
    for b in range(B):
        idx = np.nonzero(attention_mask[b])[0]
        n = len(idx)
        assert n <= K_CAP, f"unmasked key count {n} exceeds K_CAP={K_CAP}"
        idxp = np.concatenate([idx, np.zeros(K_CAP - n, np.int64)])
        # [8, 128, K_CAP] d-slabs -> range-major [128, 8*w] blocks so each
        # key-range is one contiguous DMA
        xkT = x[b][idxp].T.astype(np.float16).reshape(8, 128, K_CAP)
        xk = np.concatenate(
            [xkT[:, :, c0:c0 + w].transpose(1, 0, 2).reshape(128, 8 * w)
             for c0, w in KV_RANGES], axis=1)
        mb = np.full(K_CAP, MASKED_BIAS, np.float32)
        mb[:n] = LOGIT_SHIFT
        per_batch.append((np.ascontiguousarray(xk),
                          np.ascontiguousarray(mb.reshape(NJK, 128).T)))
    in_maps = []
    for c in range(N_CORES):
        b, h = c // 2, c % 2
        xk, mb = per_batch[b]
        # [8, 128, S_LOC] d-slabs -> pass-slab-major [128, 4 x 8 x 512] so
        # each attention pass's q input is one contiguous 1MB DMA
        xT = x[b, h * S_LOC:(h + 1) * S_LOC].T.astype(np.float16) \
            .reshape(8, 128, S_LOC)
        xq = np.concatenate(
            [xT[:, :, p * QW:(p + 1) * QW].transpose(1, 0, 2)
             .reshape(128, 8 * QW) for p in range(NP)], axis=1)
        in_maps.append({
            "xq": np.ascontiguousarray(xq),
            "xk": xk,
            "Wd": wd_c,
            "Wub": wub,
            "bd_q": bd_q,
            "bd_kv": bd_kv,
            "maskbias": mb,
        })
    return in_maps


def kernel(x, attention_mask, Wd, bd, Wu, bu):
    from concourse import bass_utils

    x = np.asarray(x, dtype=np.float32)
    attention_mask = np.asarray(attention_mask)
    Wd = np.asarray(Wd, dtype=np.float32)
    bd = np.asarray(bd, dtype=np.float32)
    Wu = np.asarray(Wu, dtype=np.float32)
    bu = np.asarray(bu, dtype=np.float32)

    nc = get_graph()
    in_maps = make_in_maps(x, attention_mask, Wd, bd, Wu, bu)
    res = bass_utils.run_bass_kernel_spmd(nc, in_maps, list(range(N_CORES)))
    out = np.empty((B, S, D), dtype=np.float32)
    for c in range(N_CORES):
        b, h = c // 2, c % 2
        out[b, h * S_LOC:(h + 1) * S_LOC, :] = \
            res.results[c]["out"].astype(np.float32)
    return out
